# revision 26
# baseline (speedup 1.0000x reference)
"""Trainium2 Bass kernel for nn_Decoder (single-step GRU decoder w/ attention).

Math notes (derived from the reference):
  - The attention branch is dead: softmax over a singleton axis yields all-ones,
    so attn_weights == 1/L exactly and context == mean(encoder_out, axis=1).
  - Real device work: (a) streaming-reduce encoder_out [B,L,D] over L
    (batch-sharded across 8 cores), (b) the vocab projection
    [B,3H] @ out_w [3H,V] (vocab-sharded across 8 cores), (c) one GRU step
    (computed on-device, replicated across cores in phase 2).

Sharding:
  phase 1: data-parallel over batch (8 batches/core) -> context mean.
  phase 2: tensor-parallel over vocab (V padded 50257->50264, 6283/core);
           each core also computes the (tiny) GRU for all 64 batches.
  Host glue between phases plays the role of the all-gather; final
  log_softmax normalization applied on host from per-shard logits.

Multi-device PJRT launches hang over the axon tunnel in this environment, so
the SPMD program is executed as 8 concurrent single-device runs (same BIR,
different per-core data) via a small runner modeled on
bass2jax.run_bass_via_pjrt's single-core branch.
"""

import hashlib
import os
import shutil
import threading
from concurrent.futures import ThreadPoolExecutor
from contextlib import ExitStack

import numpy as np

import concourse.bass as bass  # noqa: F401  (engine types pulled via nc)
import concourse.tile as tile
from concourse import bacc, mybir

# ---------------------------------------------------------------- constants
N_CORES = 8
B, L, H, E, V = 64, 2048, 512, 512, 50257
D = 2 * H                      # encoder feature dim = 1024
B_LOC = B // N_CORES           # 8 batches per core (phase 1)
K_CAT = H + D                  # 1536 contraction dim for the projection
VP = 50264                     # V padded to a multiple of 8
VS = VP // N_CORES             # 6283 vocab columns per core (phase 2)
F32 = mybir.dt.float32
BF16 = mybir.dt.bfloat16

_lock = threading.Lock()
_state: dict = {}

# Optional: test harnesses can set this to a callable(name) returning a
# context manager wrapped around each device phase (e.g. NTFF profiling).
PHASE_CTX = None


def _phase_ctx(name):
    import contextlib

    return PHASE_CTX(name) if PHASE_CTX is not None else contextlib.nullcontext()

# ------------------------------------------------------- NEFF compile cache
_NEFF_CACHE_DIR = os.path.join(
    os.path.expanduser("~"), ".cache", "bass_neff_cache"
)


def _patch_compile_cache():
    """Memoize walrus NEFF compilation by BIR hash (compile-time only)."""
    from concourse import bass2jax, bass_utils

    if getattr(bass2jax, "_neff_cache_patched", False):
        return
    orig = bass_utils.compile_bir_kernel
    lk = threading.Lock()

    def cached(bir_json, tmpdir, neff_name="file.neff"):
        key = hashlib.sha256(bir_json).hexdigest()
        path = os.path.join(_NEFF_CACHE_DIR, key + ".neff")
        with lk:
            if os.path.exists(path):
                dst = os.path.join(tmpdir, neff_name)
                shutil.copy(path, dst)
                return dst
            out = orig(bir_json, tmpdir, neff_name)
            os.makedirs(_NEFF_CACHE_DIR, exist_ok=True)
            tmp = path + f".tmp.{os.getpid()}"
            shutil.copy(out, tmp)
            os.replace(tmp, path)
            return out

    bass2jax.compile_bir_kernel = cached
    bass2jax._neff_cache_patched = True


# ----------------------------------------------------------- per-dev runner
def _make_runner(nc):
    """Single-device executor for a finalized Bass module (axon-safe)."""
    import jax
    from concourse import bass2jax

    bass2jax.install_neuronx_cc_hook()
    _patch_compile_cache()

    assert nc.partition_id_tensor is None
    in_names, out_names, out_avals, zero_specs = [], [], [], []
    for alloc in nc.m.functions[0].allocations:
        if not isinstance(alloc, mybir.MemoryLocationSet):
            continue
        name = alloc.memorylocations[0].name
        if alloc.kind == "ExternalInput":
            in_names.append(name)
        elif alloc.kind == "ExternalOutput":
            assert alloc.tensor_shape is not None and alloc.dtype is not None
            out_names.append(name)
            shape = tuple(alloc.tensor_shape)
            dtype = mybir.dt.np(alloc.dtype)
            out_avals.append(jax.core.ShapedArray(shape, dtype))
            zero_specs.append((shape, dtype))
    n_params = len(in_names)
    all_in = tuple(in_names + out_names)
    donate = tuple(range(n_params, n_params + len(out_names)))

    def _body(*args):
        outs = bass2jax._bass_exec_p.bind(
            *args,
            out_avals=tuple(out_avals),
            in_names=all_in,
            out_names=tuple(out_names),
            lowering_input_output_aliases=(),
            sim_require_finite=False,
            sim_require_nnan=False,
            nc=nc,
        )
        return tuple(outs)

    jf = jax.jit(_body, donate_argnums=donate, keep_unused=True)
    compiled_devs: set = set()
    compile_lock = threading.Lock()

    def run(dev, in_map):
        args = [
            jax.device_put(np.ascontiguousarray(np.asarray(in_map[n])), dev)
            for n in in_names
        ]
        args += [
            jax.device_put(np.zeros(s, d), dev) for (s, d) in zero_specs
        ]
        if dev not in compiled_devs:
            # serialize first-time per-device XLA compiles
            with compile_lock:
                outs = jf(*args)
                jax.block_until_ready(outs)
                compiled_devs.add(dev)
        else:
            outs = jf(*args)
            jax.block_until_ready(outs)
        return {n: np.asarray(o) for n, o in zip(out_names, outs)}

    return run


# ------------------------------------------------------------ phase 1 (ctx)
def _build_phase1():
    """Per core: ctx[b,:] = mean_l enc[b,l,:] for its 8 batches.

    enc arrives bf16 (host-cast).  Each [128, 4096] tile holds 512 L-rows
    folded 4x into the free dim; a ones-vector bf16 matmul reduces the
    partition dim and PSUM f32 accumulation folds both the tile and r axes.
    """
    nc = bacc.Bacc("TRN2", target_bir_lowering=False, debug=False, num_devices=1,
                   enable_partition_id=False)
    enc = nc.dram_tensor("enc", [B_LOC, L, D], BF16, kind="ExternalInput")
    ctx_out = nc.dram_tensor("ctx", [B_LOC, D], F32, kind="ExternalOutput")

    n_t = 4       # big tiles per batch
    n_r = 4       # L-rows folded per partition within a tile

    with tile.TileContext(nc) as tc, ExitStack() as st:
        tpool = st.enter_context(tc.tile_pool(name="enc", bufs=6))
        cpool = st.enter_context(tc.tile_pool(name="cst", bufs=1))
        ppool = st.enter_context(tc.tile_pool(name="ps", bufs=2, space="PSUM"))
        opool = st.enter_context(tc.tile_pool(name="row", bufs=2))

        ones = cpool.tile([128, 1], BF16)
        nc.gpsimd.memset(ones[:], 1.0)

        for b in range(B_LOC):
            # [2048, 1024] -> 4 tiles of [128, 4 * 1024]
            src = enc[b].rearrange("(t p r) d -> t p (r d)", p=128, r=n_r)
            ps = ppool.tile([1, D], F32)
            for t in range(n_t):
                et = tpool.tile([128, n_r * D], BF16)
                nc.sync.dma_start(et[:], src[t])
                for r in range(n_r):
                    for j in range(2):
                        c = r * D + j * 512
                        nc.tensor.matmul(
                            ps[:, j * 512:(j + 1) * 512], ones[:],
                            et[:, c:c + 512],
                            start=(t == 0 and r == 0),
                            stop=(t == n_t - 1 and r == n_r - 1))
            row = opool.tile([1, D], F32)
            nc.scalar.mul(row[:], ps[:], 1.0 / L)
            nc.sync.dma_start(ctx_out[b:b + 1, :], row[:])

    nc.compile()
    return nc


# ------------------------------------------------- phase 2 (GRU + proj)
def _build_phase2():
    """Per core: full-batch GRU step (replicated) + vocab-shard projection.

    All matmuls contract over the partition dim; activations stay f32.
    logits[:, c0:c1] = cat([h_new, ctx]) @ W_shard + out_b_shard via PSUM
    accumulation over 12 k-chunks plus a K=1 ones-row matmul for the bias.
    """
    nc = bacc.Bacc("TRN2", target_bir_lowering=False, debug=False, num_devices=1,
                   enable_partition_id=False)
    xeT = nc.dram_tensor("xeT", [E, B], BF16, kind="ExternalInput")
    h0T = nc.dram_tensor("h0T", [H, B], BF16, kind="ExternalInput")
    ctxT = nc.dram_tensor("ctxT", [D, B], BF16, kind="ExternalInput")
    h0Tf = nc.dram_tensor("h0Tf", [H, B], F32, kind="ExternalInput")
    wihT = nc.dram_tensor("wihT", [K_CAT, 3 * H], BF16, kind="ExternalInput")
    whhT = nc.dram_tensor("whhT", [H, 3 * H], BF16, kind="ExternalInput")
    brz = nc.dram_tensor("brz", [128, 8], F32, kind="ExternalInput")
    bin_ = nc.dram_tensor("bin", [128, 4], F32, kind="ExternalInput")
    bhn = nc.dram_tensor("bhn", [128, 4], F32, kind="ExternalInput")
    w_in = nc.dram_tensor("w", [K_CAT, VS], BF16, kind="ExternalInput")
    outb = nc.dram_tensor("outb", [1, VS], BF16, kind="ExternalInput")
    logits = nc.dram_tensor("logits", [B, VS], F32, kind="ExternalOutput")
    hnT_out = nc.dram_tensor("hnT", [H, B], F32, kind="ExternalOutput")

    KC = K_CAT // 128   # 12 cat-dim chunks
    KH = H // 128       # 4 hidden chunks

    with tile.TileContext(nc) as tc, ExitStack() as st:
        cpool = st.enter_context(tc.tile_pool(name="cst", bufs=1))
        gpool = st.enter_context(tc.tile_pool(name="gru", bufs=1))
        spool = st.enter_context(tc.tile_pool(name="sml", bufs=4))
        wpool = st.enter_context(tc.tile_pool(name="wts", bufs=3))
        lpool = st.enter_context(tc.tile_pool(name="lt", bufs=4))
        ps_g = st.enter_context(tc.tile_pool(name="psg", bufs=2, space="PSUM"))
        ps_p = st.enter_context(tc.tile_pool(name="psp", bufs=6, space="PSUM"))

        # --- constant / small loads
        ones1 = cpool.tile([1, B], BF16)
        nc.gpsimd.memset(ones1[:], 1.0)
        outb_sb = cpool.tile([1, VS], BF16)
        nc.sync.dma_start(outb_sb[:], outb[:])
        brz_sb = cpool.tile([128, 8], F32)
        nc.sync.dma_start(brz_sb[:], brz[:])
        bin_sb = cpool.tile([128, 4], F32)
        nc.sync.dma_start(bin_sb[:], bin_[:])
        bhn_sb = cpool.tile([128, 4], F32)
        nc.sync.dma_start(bhn_sb[:], bhn[:])

        # xt chunks: 0-3 = xe.T, 4-11 = ctx.T ; h0 chunks (all bf16)
        xt_sb = []
        for k in range(4):
            t = gpool.tile([128, B], BF16, tag=f"xt{k}")
            nc.sync.dma_start(t[:], xeT[k * 128:(k + 1) * 128, :])
            xt_sb.append(t)
        for k in range(8):
            t = gpool.tile([128, B], BF16, tag=f"ct{k}")
            nc.sync.dma_start(t[:], ctxT[k * 128:(k + 1) * 128, :])
            xt_sb.append(t)
        h0_sb = []
        for k in range(KH):
            t = gpool.tile([128, B], BF16, tag=f"h0{k}")
            nc.sync.dma_start(t[:], h0T[k * 128:(k + 1) * 128, :])
            h0_sb.append(t)
        # f32 copy of h0 for the elementwise h_new update (bf16 only feeds PE)
        h0f_sb = []
        for k in range(KH):
            t = gpool.tile([128, B], F32, tag=f"h0f{k}")
            nc.sync.dma_start(t[:], h0Tf[k * 128:(k + 1) * 128, :])
            h0f_sb.append(t)

        wih_sb = []
        for k in range(KC):
            t = gpool.tile([128, 3 * H], BF16, tag=f"wi{k}")
            nc.sync.dma_start(t[:], wihT[k * 128:(k + 1) * 128, :])
            wih_sb.append(t)
        whh_sb = []
        for k in range(KH):
            t = gpool.tile([128, 3 * H], BF16, tag=f"wh{k}")
            nc.sync.dma_start(t[:], whhT[k * 128:(k + 1) * 128, :])
            whh_sb.append(t)

        # --- GRU gates.  r/z: sigma(gi + gh + b); chunks g=0..7 of 3H rows.
        Sig = mybir.ActivationFunctionType.Sigmoid
        Ident = mybir.ActivationFunctionType.Identity
        TanhF = mybir.ActivationFunctionType.Tanh
        rz_sb = []
        for g in range(8):
            ps = ps_g.tile([128, B], F32, tag="psg")
            for k in range(KC):
                nc.tensor.matmul(ps[:], wih_sb[k][:, g * 128:(g + 1) * 128],
                                 xt_sb[k][:], start=(k == 0), stop=False)
            for k in range(KH):
                nc.tensor.matmul(ps[:], whh_sb[k][:, g * 128:(g + 1) * 128],
                                 h0_sb[k][:], start=False, stop=(k == KH - 1))
            act = gpool.tile([128, B], F32, tag=f"rz{g}")
            nc.scalar.activation(act[:], ps[:], Sig, bias=brz_sb[:, g:g + 1])
            rz_sb.append(act)

        # n chunks j=0..3 (rows 2H..3H) and h_new
        hn_new = []
        for j in range(KH):
            g = 8 + j
            ps_in = ps_g.tile([128, B], F32, tag="psg")
            for k in range(KC):
                nc.tensor.matmul(ps_in[:], wih_sb[k][:, g * 128:(g + 1) * 128],
                                 xt_sb[k][:], start=(k == 0), stop=(k == KC - 1))
            ps_hn = ps_g.tile([128, B], F32, tag="psg")
            for k in range(KH):
                nc.tensor.matmul(ps_hn[:], whh_sb[k][:, g * 128:(g + 1) * 128],
                                 h0_sb[k][:], start=(k == 0), stop=(k == KH - 1))
            in_sb = spool.tile([128, B], F32, tag="t_in")
            nc.scalar.activation(in_sb[:], ps_in[:], Ident,
                                 bias=bin_sb[:, j:j + 1])
            hn_sb = spool.tile([128, B], F32, tag="t_hn")
            nc.scalar.activation(hn_sb[:], ps_hn[:], Ident,
                                 bias=bhn_sb[:, j:j + 1])
            rhn = spool.tile([128, B], F32, tag="t_rhn")
            nc.vector.tensor_mul(rhn[:], rz_sb[j][:], hn_sb[:])
            pre_n = spool.tile([128, B], F32, tag="t_pre")
            nc.vector.tensor_add(pre_n[:], in_sb[:], rhn[:])
            n_sb = spool.tile([128, B], F32, tag="t_n")
            nc.scalar.activation(n_sb[:], pre_n[:], TanhF)
            d_sb = spool.tile([128, B], F32, tag="t_d")
            nc.vector.tensor_sub(d_sb[:], h0f_sb[j][:], n_sb[:])
            zd = spool.tile([128, B], F32, tag="t_zd")
            nc.vector.tensor_mul(zd[:], rz_sb[4 + j][:], d_sb[:])
            hnw = gpool.tile([128, B], F32, tag=f"hn{j}")
            nc.vector.tensor_add(hnw[:], n_sb[:], zd[:])
            nc.sync.dma_start(hnT_out[j * 128:(j + 1) * 128, :], hnw[:])
            hnb = gpool.tile([128, B], BF16, tag=f"hnb{j}")
            nc.vector.tensor_copy(hnb[:], hnw[:])
            hn_new.append(hnb)

        cat_sb = hn_new + xt_sb[4:]  # 12 bf16 chunks [128, B] = [h_new; ctx].T

        # --- vocab projection: bf16, 512-wide vtiles (one PSUM bank each),
        #     groups of <=6 (6 banks + 2 GRU = 8)
        bounds = [0, 2560, 5120, VS]
        groups = []
        for gi in range(len(bounds) - 1):
            c0, c1 = bounds[gi], bounds[gi + 1]
            vt = [(c0 + v * 512, min(c0 + (v + 1) * 512, c1))
                  for v in range((c1 - c0 + 511) // 512)]
            groups.append((c0, c1, vt))

        for (c0, c1, vtiles) in groups:
            gw = c1 - c0
            wk_tiles = []
            for k in range(KC):
                wk = wpool.tile([128, gw], BF16, tag="wk")
                nc.sync.dma_start(wk[:], w_in[k * 128:(k + 1) * 128, c0:c1])
                wk_tiles.append(wk)
            ps_list = []
            for (v0, v1) in vtiles:
                ps = ps_p.tile([B, 512], F32, tag="psp")
                ps_list.append(ps)
            for k in range(KC):
                for i, (v0, v1) in enumerate(vtiles):
                    nc.tensor.matmul(
                        ps_list[i][:, 0:v1 - v0], cat_sb[k][:],
                        wk_tiles[k][:, v0 - c0:v1 - c0],
                        start=(k == 0), stop=False)
            for i, (v0, v1) in enumerate(vtiles):
                nc.tensor.matmul(ps_list[i][:, 0:v1 - v0], ones1[:],
                                 outb_sb[:, v0:v1], start=False, stop=True)
                lt = lpool.tile([B, 512], F32, tag="lt")
                nc.vector.tensor_copy(lt[:, 0:v1 - v0], ps_list[i][:, 0:v1 - v0])
                nc.sync.dma_start(logits[:, v0:v1], lt[:, 0:v1 - v0])

    nc.compile()
    return nc


# ------------------------------------------------------------------ driver
def _get_state():
    import jax

    with _lock:
        if "p1" not in _state:
            _state["p1"] = _build_phase1()
            _state["run1"] = _make_runner(_state["p1"])
            _state["p2"] = _build_phase2()
            _state["run2"] = _make_runner(_state["p2"])
            # First transfer in a process pays a large one-time session
            # cost on the remote side; warm it with a tiny put per device.
            tiny = np.zeros(8, np.float32)
            for d in jax.devices()[:N_CORES]:
                jax.device_put(tiny, d).block_until_ready()
        return _state


def _run_cores(run, in_maps, devs):
    with ThreadPoolExecutor(N_CORES) as ex:
        futs = [ex.submit(run, devs[c], in_maps[c]) for c in range(N_CORES)]
        return [f.result() for f in futs]


def kernel(x, h, encoder_out, use_cuda=None, emb=None, attn_w=None,
           attn_b=None, w_ih=None, w_hh=None, b_ih=None, b_hh=None,
           out_w=None, out_b=None):
    import jax

    x = np.asarray(x)
    h = np.asarray(h, dtype=np.float32)
    encoder_out = np.ascontiguousarray(np.asarray(encoder_out, dtype=np.float32))
    emb = np.asarray(emb, dtype=np.float32)
    w_ih = np.asarray(w_ih, dtype=np.float32)
    w_hh = np.asarray(w_hh, dtype=np.float32)
    b_ih = np.asarray(b_ih, dtype=np.float32)
    b_hh = np.asarray(b_hh, dtype=np.float32)
    out_w = np.asarray(out_w, dtype=np.float32)
    out_b = np.asarray(out_b, dtype=np.float32)

    import ml_dtypes

    bf16 = ml_dtypes.bfloat16
    st = _get_state()
    devs = jax.devices()[:N_CORES]

    # ---------------- phase 1: context mean, batch-sharded (bf16 stream)
    enc_bf = encoder_out.astype(bf16)
    in1 = [{"enc": enc_bf[c * B_LOC:(c + 1) * B_LOC]} for c in range(N_CORES)]
    with _phase_ctx("p1"):
        res1 = _run_cores(st["run1"], in1, devs)
    context = np.concatenate([r["ctx"] for r in res1], axis=0)  # [B, D] f32

    # ---------------- host glue (the "all-gather")
    xe = emb[np.asarray(x[:, 0], dtype=np.int64)]               # [B, E]
    xeT = np.ascontiguousarray(xe.T).astype(bf16)
    h0Tf = np.ascontiguousarray(h[0].T)                         # [H, B] f32
    h0T = h0Tf.astype(bf16)
    ctxT = np.ascontiguousarray(context.T).astype(bf16)         # [D, B]
    wihT = np.ascontiguousarray(w_ih.T).astype(bf16)            # [K_CAT, 3H]
    whhT = np.ascontiguousarray(w_hh.T).astype(bf16)            # [H, 3H]
    bsum = b_ih + b_hh
    brz = np.ascontiguousarray(bsum[:2 * H].reshape(8, 128).T)
    bin_ = np.ascontiguousarray(b_ih[2 * H:].reshape(4, 128).T)
    bhn = np.ascontiguousarray(b_hh[2 * H:].reshape(4, 128).T)

    w_pad = np.zeros((K_CAT, VP), dtype=bf16)
    w_pad[:, :V] = out_w.astype(bf16)
    b_pad = np.zeros((1, VP), dtype=bf16)
    b_pad[0, :V] = out_b.astype(bf16)

    # ---------------- phase 2: GRU + vocab-sharded projection
    in2 = []
    for c in range(N_CORES):
        sl = slice(c * VS, (c + 1) * VS)
        in2.append({
            "xeT": xeT, "h0T": h0T, "h0Tf": h0Tf, "ctxT": ctxT,
            "wihT": wihT, "whhT": whhT, "brz": brz, "bin": bin_, "bhn": bhn,
            "w": np.ascontiguousarray(w_pad[:, sl]),
            "outb": np.ascontiguousarray(b_pad[:, sl]),
        })
    with _phase_ctx("p2"):
        res2 = _run_cores(st["run2"], in2, devs)

    logits = np.concatenate([r["logits"] for r in res2], axis=1)[:, :V]
    h_new = np.ascontiguousarray(res2[0]["hnT"].T)              # [B, H]

    # ---------------- host epilogue: log_softmax normalization
    lg64 = logits.astype(np.float64)
    m = lg64.max(axis=1, keepdims=True)
    lse = (m + np.log(np.exp(lg64 - m).sum(axis=1, keepdims=True)))
    logp = (lg64 - lse).astype(np.float32)

    hidden = h_new[None]                                        # [1, B, H]
    attn_weights = np.full((B, 1, L), np.float32(1.0) / np.float32(L),
                           dtype=np.float32)
    return logp, hidden, attn_weights


# revision 40
# speedup vs baseline: 28325.5428x; 28325.5428x over previous
"""Trainium2 Bass kernel for nn_Decoder (single-step GRU decoder w/ attention).

Math notes (derived from the reference):
  - The attention branch is dead: softmax over a singleton axis yields all-ones,
    so attn_weights == 1/L exactly and context == mean(encoder_out, axis=1).
  - Real device work: (a) streaming-reduce encoder_out [B,L,D] over L
    (batch-sharded across 8 cores), (b) the vocab projection
    [B,3H] @ out_w [3H,V] (vocab-sharded across 8 cores), (c) one GRU step
    (computed on-device, replicated across cores in phase 2).

Sharding:
  phase 1: data-parallel over batch (8 batches/core) -> context mean.
  phase 2: tensor-parallel over vocab (V padded 50257->50264, 6283/core);
           each core also computes the (tiny) GRU for all 64 batches.
  Host glue between phases plays the role of the all-gather; final
  log_softmax normalization applied on host from per-shard logits.

Multi-device PJRT launches hang over the axon tunnel in this environment, so
the SPMD program is executed as 8 concurrent single-device runs (same BIR,
different per-core data) via a small runner modeled on
bass2jax.run_bass_via_pjrt's single-core branch.
"""

import hashlib
import os
import shutil
import threading
from concurrent.futures import ThreadPoolExecutor
from contextlib import ExitStack

import numpy as np

import concourse.bass as bass  # noqa: F401  (engine types pulled via nc)
import concourse.tile as tile
from concourse import bacc, mybir

# ---------------------------------------------------------------- constants
N_CORES = 8
B, L, H, E, V = 64, 2048, 512, 512, 50257
D = 2 * H                      # encoder feature dim = 1024
B_LOC = B // N_CORES           # 8 batches per core (phase 1)
K_CAT = H + D                  # 1536 contraction dim for the projection
VP = 50264                     # V padded to a multiple of 8
VS = VP // N_CORES             # 6283 vocab columns per core (phase 2)
F32 = mybir.dt.float32
BF16 = mybir.dt.bfloat16

_lock = threading.Lock()
_state: dict = {}

# Optional: test harnesses can set this to a callable(name) returning a
# context manager wrapped around each device phase (e.g. NTFF profiling).
PHASE_CTX = None


def _phase_ctx(name):
    import contextlib

    return PHASE_CTX(name) if PHASE_CTX is not None else contextlib.nullcontext()

# ------------------------------------------------------- NEFF compile cache
_NEFF_CACHE_DIR = os.path.join(
    os.path.expanduser("~"), ".cache", "bass_neff_cache"
)


def _patch_compile_cache():
    """Memoize walrus NEFF compilation by BIR hash (compile-time only)."""
    from concourse import bass2jax, bass_utils

    if getattr(bass2jax, "_neff_cache_patched", False):
        return
    orig = bass_utils.compile_bir_kernel
    lk = threading.Lock()

    def cached(bir_json, tmpdir, neff_name="file.neff"):
        key = hashlib.sha256(bir_json).hexdigest()
        path = os.path.join(_NEFF_CACHE_DIR, key + ".neff")
        with lk:
            if os.path.exists(path):
                dst = os.path.join(tmpdir, neff_name)
                shutil.copy(path, dst)
                return dst
            out = orig(bir_json, tmpdir, neff_name)
            os.makedirs(_NEFF_CACHE_DIR, exist_ok=True)
            tmp = path + f".tmp.{os.getpid()}"
            shutil.copy(out, tmp)
            os.replace(tmp, path)
            return out

    bass2jax.compile_bir_kernel = cached
    bass2jax._neff_cache_patched = True


# ----------------------------------------------------------- per-dev runner
def _make_runner(nc):
    """Single-device executor for a finalized Bass module (axon-safe)."""
    import jax
    from concourse import bass2jax

    bass2jax.install_neuronx_cc_hook()
    _patch_compile_cache()

    assert nc.partition_id_tensor is None
    in_names, out_names, out_avals, zero_specs = [], [], [], []
    for alloc in nc.m.functions[0].allocations:
        if not isinstance(alloc, mybir.MemoryLocationSet):
            continue
        name = alloc.memorylocations[0].name
        if alloc.kind == "ExternalInput":
            in_names.append(name)
        elif alloc.kind == "ExternalOutput":
            assert alloc.tensor_shape is not None and alloc.dtype is not None
            out_names.append(name)
            shape = tuple(alloc.tensor_shape)
            dtype = mybir.dt.np(alloc.dtype)
            out_avals.append(jax.core.ShapedArray(shape, dtype))
            zero_specs.append((shape, dtype))
    n_params = len(in_names)
    all_in = tuple(in_names + out_names)
    donate = tuple(range(n_params, n_params + len(out_names)))

    def _body(*args):
        outs = bass2jax._bass_exec_p.bind(
            *args,
            out_avals=tuple(out_avals),
            in_names=all_in,
            out_names=tuple(out_names),
            lowering_input_output_aliases=(),
            sim_require_finite=False,
            sim_require_nnan=False,
            nc=nc,
        )
        return tuple(outs)

    jf = jax.jit(_body, donate_argnums=donate, keep_unused=True)
    compiled_devs: set = set()
    compile_lock = threading.Lock()

    def run(dev, in_map):
        args = [
            jax.device_put(np.ascontiguousarray(np.asarray(in_map[n])), dev)
            for n in in_names
        ]
        args += [
            jax.device_put(np.zeros(s, d), dev) for (s, d) in zero_specs
        ]
        if dev not in compiled_devs:
            # serialize first-time per-device XLA compiles
            with compile_lock:
                outs = jf(*args)
                jax.block_until_ready(outs)
                compiled_devs.add(dev)
        else:
            outs = jf(*args)
            jax.block_until_ready(outs)
        return {n: np.asarray(o) for n, o in zip(out_names, outs)}

    return run


# ------------------------------------------------------------ phase 1 (ctx)
def _build_phase1():
    """Per core: ctx[b,:] = mean_l enc[b,l,:] for its 8 batches.

    enc arrives bf16 (host-cast).  Each [128, 4096] tile holds 512 L-rows
    folded 4x into the free dim; a ones-vector bf16 matmul reduces the
    partition dim and PSUM f32 accumulation folds both the tile and r axes.
    """
    nc = bacc.Bacc("TRN2", target_bir_lowering=False, debug=False, num_devices=1,
                   enable_partition_id=False)
    enc = nc.dram_tensor("enc", [B_LOC, L, D], BF16, kind="ExternalInput")
    ctx_out = nc.dram_tensor("ctx", [B_LOC, D], F32, kind="ExternalOutput")

    n_t = 4       # big tiles per batch
    n_r = 4       # L-rows folded per partition within a tile

    with tile.TileContext(nc) as tc, ExitStack() as st:
        tpool = st.enter_context(tc.tile_pool(name="enc", bufs=6))
        cpool = st.enter_context(tc.tile_pool(name="cst", bufs=1))
        ppool = st.enter_context(tc.tile_pool(name="ps", bufs=2, space="PSUM"))
        opool = st.enter_context(tc.tile_pool(name="row", bufs=2))

        ones = cpool.tile([128, 1], BF16)
        nc.gpsimd.memset(ones[:], 1.0)

        for b in range(B_LOC):
            # [2048, 1024] -> 4 tiles of [128, 4 * 1024]
            src = enc[b].rearrange("(t p r) d -> t p (r d)", p=128, r=n_r)
            ps = ppool.tile([1, D], F32)
            for t in range(n_t):
                et = tpool.tile([128, n_r * D], BF16)
                nc.sync.dma_start(et[:], src[t])
                for r in range(n_r):
                    for j in range(2):
                        c = r * D + j * 512
                        nc.tensor.matmul(
                            ps[:, j * 512:(j + 1) * 512], ones[:],
                            et[:, c:c + 512],
                            start=(t == 0 and r == 0),
                            stop=(t == n_t - 1 and r == n_r - 1))
            row = opool.tile([1, D], F32)
            nc.scalar.mul(row[:], ps[:], 1.0 / L)
            nc.sync.dma_start(ctx_out[b:b + 1, :], row[:])

    nc.compile()
    return nc


# ------------------------------------------------- phase 2 (GRU + proj)
def _build_phase2():
    """Per core: full-batch GRU step (replicated) + vocab-shard projection.

    All matmuls contract over the partition dim; activations stay f32.
    logits[:, c0:c1] = cat([h_new, ctx]) @ W_shard + out_b_shard via PSUM
    accumulation over 12 k-chunks plus a K=1 ones-row matmul for the bias.
    """
    nc = bacc.Bacc("TRN2", target_bir_lowering=False, debug=False, num_devices=1,
                   enable_partition_id=False)
    F32R = mybir.dt.float32r
    xeT = nc.dram_tensor("xeT", [E, B], BF16, kind="ExternalInput")
    ctxT = nc.dram_tensor("ctxT", [D, B], BF16, kind="ExternalInput")
    h0Tf = nc.dram_tensor("h0Tf", [H, B], F32, kind="ExternalInput")
    wihT = nc.dram_tensor("wihT", [K_CAT, 3 * H], BF16, kind="ExternalInput")
    whhT = nc.dram_tensor("whhT", [H, 3 * H], F32, kind="ExternalInput")
    brz = nc.dram_tensor("brz", [128, 8], F32, kind="ExternalInput")
    bin_ = nc.dram_tensor("bin", [128, 4], F32, kind="ExternalInput")
    bhn = nc.dram_tensor("bhn", [128, 4], F32, kind="ExternalInput")
    w_in = nc.dram_tensor("w", [K_CAT, VS], BF16, kind="ExternalInput")
    outb = nc.dram_tensor("outb", [1, VS], BF16, kind="ExternalInput")
    logits = nc.dram_tensor("logits", [B, VS], F32, kind="ExternalOutput")
    hnT_out = nc.dram_tensor("hnT", [H, B], F32, kind="ExternalOutput")

    KC = K_CAT // 128   # 12 cat-dim chunks
    KH = H // 128       # 4 hidden chunks

    with tile.TileContext(nc) as tc, ExitStack() as st:
        cpool = st.enter_context(tc.tile_pool(name="cst", bufs=1))
        gpool = st.enter_context(tc.tile_pool(name="gru", bufs=1))
        spool = st.enter_context(tc.tile_pool(name="sml", bufs=4))
        wpool = st.enter_context(tc.tile_pool(name="wts", bufs=3))
        lpool = st.enter_context(tc.tile_pool(name="lt", bufs=4))
        ps_g = st.enter_context(tc.tile_pool(name="psg", bufs=2, space="PSUM"))
        ps_p = st.enter_context(tc.tile_pool(name="psp", bufs=6, space="PSUM"))

        # --- constant / small loads
        ones1 = cpool.tile([1, B], BF16)
        nc.gpsimd.memset(ones1[:], 1.0)
        outb_sb = cpool.tile([1, VS], BF16)
        nc.sync.dma_start(outb_sb[:], outb[:])
        brz_sb = cpool.tile([128, 8], F32)
        nc.sync.dma_start(brz_sb[:], brz[:])
        bin_sb = cpool.tile([128, 4], F32)
        nc.sync.dma_start(bin_sb[:], bin_[:])
        bhn_sb = cpool.tile([128, 4], F32)
        nc.sync.dma_start(bhn_sb[:], bhn[:])

        # xt chunks: 0-3 = xe.T, 4-11 = ctx.T ; h0 chunks (all bf16)
        xt_sb = []
        for k in range(4):
            t = gpool.tile([128, B], BF16, tag=f"xt{k}")
            nc.sync.dma_start(t[:], xeT[k * 128:(k + 1) * 128, :])
            xt_sb.append(t)
        for k in range(8):
            t = gpool.tile([128, B], BF16, tag=f"ct{k}")
            nc.sync.dma_start(t[:], ctxT[k * 128:(k + 1) * 128, :])
            xt_sb.append(t)
        # h0 in f32: feeds both the (exact f32) gh matmuls and the
        # elementwise h_new update.
        h0f_sb = []
        for k in range(KH):
            t = gpool.tile([128, B], F32, tag=f"h0f{k}")
            nc.sync.dma_start(t[:], h0Tf[k * 128:(k + 1) * 128, :])
            h0f_sb.append(t)
        h0_sb = [t[:] for t in h0f_sb]

        wih_sb = []
        for k in range(KC):
            t = gpool.tile([128, 3 * H], BF16, tag=f"wi{k}")
            nc.sync.dma_start(t[:], wihT[k * 128:(k + 1) * 128, :])
            wih_sb.append(t)
        whh_sb = []
        for k in range(KH):
            t = gpool.tile([128, 3 * H], F32, tag=f"wh{k}")
            nc.sync.dma_start(t[:], whhT[k * 128:(k + 1) * 128, :])
            whh_sb.append(t)

        # --- GRU gates.  r/z: sigma(gi + gh + b); chunks g=0..7 of 3H rows.
        Sig = mybir.ActivationFunctionType.Sigmoid
        Ident = mybir.ActivationFunctionType.Identity
        TanhF = mybir.ActivationFunctionType.Tanh
        rz_sb = []
        for g in range(8):
            ps = ps_g.tile([128, B], F32, tag="psg")
            for k in range(KC):
                nc.tensor.matmul(ps[:], wih_sb[k][:, g * 128:(g + 1) * 128],
                                 xt_sb[k][:], start=(k == 0), stop=False)
            for k in range(KH):
                nc.tensor.matmul(
                    ps[:], whh_sb[k][:, g * 128:(g + 1) * 128],
                    h0_sb[k], start=False, stop=(k == KH - 1))
            act = gpool.tile([128, B], F32, tag=f"rz{g}")
            nc.scalar.activation(act[:], ps[:], Sig, bias=brz_sb[:, g:g + 1])
            rz_sb.append(act)

        # n chunks j=0..3 (rows 2H..3H) and h_new
        hn_new = []
        for j in range(KH):
            g = 8 + j
            ps_in = ps_g.tile([128, B], F32, tag="psg")
            for k in range(KC):
                nc.tensor.matmul(ps_in[:], wih_sb[k][:, g * 128:(g + 1) * 128],
                                 xt_sb[k][:], start=(k == 0), stop=(k == KC - 1))
            ps_hn = ps_g.tile([128, B], F32, tag="psg")
            for k in range(KH):
                nc.tensor.matmul(
                    ps_hn[:], whh_sb[k][:, g * 128:(g + 1) * 128],
                    h0_sb[k], start=(k == 0), stop=(k == KH - 1))
            in_sb = spool.tile([128, B], F32, tag="t_in")
            nc.scalar.activation(in_sb[:], ps_in[:], Ident,
                                 bias=bin_sb[:, j:j + 1])
            hn_sb = spool.tile([128, B], F32, tag="t_hn")
            nc.scalar.activation(hn_sb[:], ps_hn[:], Ident,
                                 bias=bhn_sb[:, j:j + 1])
            rhn = spool.tile([128, B], F32, tag="t_rhn")
            nc.vector.tensor_mul(rhn[:], rz_sb[j][:], hn_sb[:])
            pre_n = spool.tile([128, B], F32, tag="t_pre")
            nc.vector.tensor_add(pre_n[:], in_sb[:], rhn[:])
            n_sb = spool.tile([128, B], F32, tag="t_n")
            nc.scalar.activation(n_sb[:], pre_n[:], TanhF)
            d_sb = spool.tile([128, B], F32, tag="t_d")
            nc.vector.tensor_sub(d_sb[:], h0f_sb[j][:], n_sb[:])
            zd = spool.tile([128, B], F32, tag="t_zd")
            nc.vector.tensor_mul(zd[:], rz_sb[4 + j][:], d_sb[:])
            hnw = gpool.tile([128, B], F32, tag=f"hn{j}")
            nc.vector.tensor_add(hnw[:], n_sb[:], zd[:])
            nc.sync.dma_start(hnT_out[j * 128:(j + 1) * 128, :], hnw[:])
            hnb = gpool.tile([128, B], BF16, tag=f"hnb{j}")
            nc.vector.tensor_copy(hnb[:], hnw[:])
            hn_new.append(hnb)

        cat_sb = hn_new + xt_sb[4:]  # 12 bf16 chunks [128, B] = [h_new; ctx].T

        # --- vocab projection: bf16, 512-wide vtiles (one PSUM bank each),
        #     groups of <=6 (6 banks + 2 GRU = 8)
        bounds = [0, 2560, 5120, VS]
        groups = []
        for gi in range(len(bounds) - 1):
            c0, c1 = bounds[gi], bounds[gi + 1]
            vt = [(c0 + v * 512, min(c0 + (v + 1) * 512, c1))
                  for v in range((c1 - c0 + 511) // 512)]
            groups.append((c0, c1, vt))

        for (c0, c1, vtiles) in groups:
            gw = c1 - c0
            wk_tiles = []
            for k in range(KC):
                wk = wpool.tile([128, gw], BF16, tag="wk")
                nc.sync.dma_start(wk[:], w_in[k * 128:(k + 1) * 128, c0:c1])
                wk_tiles.append(wk)
            ps_list = []
            for (v0, v1) in vtiles:
                ps = ps_p.tile([B, 512], F32, tag="psp")
                ps_list.append(ps)
            for k in range(KC):
                for i, (v0, v1) in enumerate(vtiles):
                    nc.tensor.matmul(
                        ps_list[i][:, 0:v1 - v0], cat_sb[k][:],
                        wk_tiles[k][:, v0 - c0:v1 - c0],
                        start=(k == 0), stop=False)
            for i, (v0, v1) in enumerate(vtiles):
                nc.tensor.matmul(ps_list[i][:, 0:v1 - v0], ones1[:],
                                 outb_sb[:, v0:v1], start=False, stop=True)
                lt = lpool.tile([B, 512], F32, tag="lt")
                nc.vector.tensor_copy(lt[:, 0:v1 - v0], ps_list[i][:, 0:v1 - v0])
                nc.sync.dma_start(logits[:, v0:v1], lt[:, 0:v1 - v0])

    nc.compile()
    return nc


# ------------------------------------------------------------------ driver
def _get_state():
    import jax

    with _lock:
        if "p1" not in _state:
            _state["p1"] = _build_phase1()
            _state["run1"] = _make_runner(_state["p1"])
            _state["p2"] = _build_phase2()
            _state["run2"] = _make_runner(_state["p2"])
            # First transfer in a process pays a large one-time session
            # cost on the remote side; warm it with a tiny put per device.
            tiny = np.zeros(8, np.float32)
            for d in jax.devices()[:N_CORES]:
                jax.device_put(tiny, d).block_until_ready()
        return _state


def _run_cores(run, in_maps, devs):
    with ThreadPoolExecutor(N_CORES) as ex:
        futs = [ex.submit(run, devs[c], in_maps[c]) for c in range(N_CORES)]
        return [f.result() for f in futs]


def kernel(x, h, encoder_out, use_cuda=None, emb=None, attn_w=None,
           attn_b=None, w_ih=None, w_hh=None, b_ih=None, b_hh=None,
           out_w=None, out_b=None):
    import jax

    x = np.asarray(x)
    h = np.asarray(h, dtype=np.float32)
    encoder_out = np.ascontiguousarray(np.asarray(encoder_out, dtype=np.float32))
    emb = np.asarray(emb, dtype=np.float32)
    w_ih = np.asarray(w_ih, dtype=np.float32)
    w_hh = np.asarray(w_hh, dtype=np.float32)
    b_ih = np.asarray(b_ih, dtype=np.float32)
    b_hh = np.asarray(b_hh, dtype=np.float32)
    out_w = np.asarray(out_w, dtype=np.float32)
    out_b = np.asarray(out_b, dtype=np.float32)

    import ml_dtypes

    bf16 = ml_dtypes.bfloat16
    st = _get_state()
    devs = jax.devices()[:N_CORES]

    # ---------------- phase 1: context mean, batch-sharded (bf16 stream)
    enc_bf = encoder_out.astype(bf16)
    in1 = [{"enc": enc_bf[c * B_LOC:(c + 1) * B_LOC]} for c in range(N_CORES)]
    with _phase_ctx("p1"):
        res1 = _run_cores(st["run1"], in1, devs)
    context = np.concatenate([r["ctx"] for r in res1], axis=0)  # [B, D] f32

    # ---------------- host glue (the "all-gather")
    xe = emb[np.asarray(x[:, 0], dtype=np.int64)]               # [B, E]
    xeT = np.ascontiguousarray(xe.T).astype(bf16)
    h0Tf = np.ascontiguousarray(h[0].T)                         # [H, B] f32
    ctxT = np.ascontiguousarray(context.T).astype(bf16)         # [D, B]
    wihT = np.ascontiguousarray(w_ih.T).astype(bf16)            # [K_CAT, 3H]
    whhT = np.ascontiguousarray(w_hh.T)                         # [H, 3H] f32
    bsum = b_ih + b_hh
    brz = np.ascontiguousarray(bsum[:2 * H].reshape(8, 128).T)
    bin_ = np.ascontiguousarray(b_ih[2 * H:].reshape(4, 128).T)
    bhn = np.ascontiguousarray(b_hh[2 * H:].reshape(4, 128).T)

    w_pad = np.zeros((K_CAT, VP), dtype=bf16)
    w_pad[:, :V] = out_w.astype(bf16)
    b_pad = np.zeros((1, VP), dtype=bf16)
    b_pad[0, :V] = out_b.astype(bf16)

    # ---------------- phase 2: GRU + vocab-sharded projection
    in2 = []
    for c in range(N_CORES):
        sl = slice(c * VS, (c + 1) * VS)
        in2.append({
            "xeT": xeT, "h0T": h0Tf, "h0Tf": h0Tf, "ctxT": ctxT,
            "wihT": wihT, "whhT": whhT, "brz": brz, "bin": bin_, "bhn": bhn,
            "w": np.ascontiguousarray(w_pad[:, sl]),
            "outb": np.ascontiguousarray(b_pad[:, sl]),
        })
    with _phase_ctx("p2"):
        res2 = _run_cores(st["run2"], in2, devs)

    logits = np.concatenate([r["logits"] for r in res2], axis=1)[:, :V]
    h_new = np.ascontiguousarray(res2[0]["hnT"].T)              # [B, H]

    # ---------------- host epilogue: log_softmax normalization
    lg64 = logits.astype(np.float64)
    m = lg64.max(axis=1, keepdims=True)
    lse = (m + np.log(np.exp(lg64 - m).sum(axis=1, keepdims=True)))
    logp = (lg64 - lse).astype(np.float32)

    hidden = h_new[None]                                        # [1, B, H]
    attn_weights = np.full((B, 1, L), np.float32(1.0) / np.float32(L),
                           dtype=np.float32)
    return logp, hidden, attn_weights


# revision 45
# speedup vs baseline: 36558.6975x; 1.2907x over previous
"""Trainium2 Bass kernel for nn_Decoder (single-step GRU decoder w/ attention).

Math notes (derived from the reference):
  - The attention branch is dead: softmax over a singleton axis yields all-ones,
    so attn_weights == 1/L exactly and context == mean(encoder_out, axis=1).
  - Real device work: (a) streaming-reduce encoder_out [B,L,D] over L
    (batch-sharded across 8 cores), (b) the vocab projection
    [B,3H] @ out_w [3H,V] (vocab-sharded across 8 cores), (c) one GRU step
    (computed on-device, replicated across cores in phase 2).

Sharding:
  phase 1: data-parallel over batch (8 batches/core) -> context mean.
  phase 2: tensor-parallel over vocab (V padded 50257->50264, 6283/core);
           each core also computes the (tiny) GRU for all 64 batches.
  Host glue between phases plays the role of the all-gather; final
  log_softmax normalization applied on host from per-shard logits.

Multi-device PJRT launches hang over the axon tunnel in this environment, so
the SPMD program is executed as 8 concurrent single-device runs (same BIR,
different per-core data) via a small runner modeled on
bass2jax.run_bass_via_pjrt's single-core branch.
"""

import hashlib
import os
import shutil
import threading
from concurrent.futures import ThreadPoolExecutor
from contextlib import ExitStack

import numpy as np

import concourse.bass as bass  # noqa: F401  (engine types pulled via nc)
import concourse.tile as tile
from concourse import bacc, mybir

# ---------------------------------------------------------------- constants
N_CORES = 8
B, L, H, E, V = 64, 2048, 512, 512, 50257
D = 2 * H                      # encoder feature dim = 1024
B_LOC = B // N_CORES           # 8 batches per core (phase 1)
K_CAT = H + D                  # 1536 contraction dim for the projection
VP = 50264                     # V padded to a multiple of 8
VS = VP // N_CORES             # 6283 vocab columns per core (phase 2)
F32 = mybir.dt.float32
BF16 = mybir.dt.bfloat16
FP8 = mybir.dt.float8e4

_lock = threading.Lock()
_state: dict = {}

# Optional: test harnesses can set this to a callable(name) returning a
# context manager wrapped around each device phase (e.g. NTFF profiling).
PHASE_CTX = None


def _phase_ctx(name):
    import contextlib

    return PHASE_CTX(name) if PHASE_CTX is not None else contextlib.nullcontext()

# ------------------------------------------------------- NEFF compile cache
_NEFF_CACHE_DIR = os.path.join(
    os.path.expanduser("~"), ".cache", "bass_neff_cache"
)


def _patch_compile_cache():
    """Memoize walrus NEFF compilation by BIR hash (compile-time only)."""
    from concourse import bass2jax, bass_utils

    if getattr(bass2jax, "_neff_cache_patched", False):
        return
    orig = bass_utils.compile_bir_kernel
    lk = threading.Lock()

    def cached(bir_json, tmpdir, neff_name="file.neff"):
        key = hashlib.sha256(bir_json).hexdigest()
        path = os.path.join(_NEFF_CACHE_DIR, key + ".neff")
        with lk:
            if os.path.exists(path):
                dst = os.path.join(tmpdir, neff_name)
                shutil.copy(path, dst)
                return dst
            out = orig(bir_json, tmpdir, neff_name)
            os.makedirs(_NEFF_CACHE_DIR, exist_ok=True)
            tmp = path + f".tmp.{os.getpid()}"
            shutil.copy(out, tmp)
            os.replace(tmp, path)
            return out

    bass2jax.compile_bir_kernel = cached
    bass2jax._neff_cache_patched = True


# ----------------------------------------------------------- per-dev runner
def _make_runner(nc):
    """Single-device executor for a finalized Bass module (axon-safe)."""
    import jax
    from concourse import bass2jax

    bass2jax.install_neuronx_cc_hook()
    _patch_compile_cache()

    assert nc.partition_id_tensor is None
    in_names, out_names, out_avals, zero_specs = [], [], [], []
    for alloc in nc.m.functions[0].allocations:
        if not isinstance(alloc, mybir.MemoryLocationSet):
            continue
        name = alloc.memorylocations[0].name
        if alloc.kind == "ExternalInput":
            in_names.append(name)
        elif alloc.kind == "ExternalOutput":
            assert alloc.tensor_shape is not None and alloc.dtype is not None
            out_names.append(name)
            shape = tuple(alloc.tensor_shape)
            dtype = mybir.dt.np(alloc.dtype)
            out_avals.append(jax.core.ShapedArray(shape, dtype))
            zero_specs.append((shape, dtype))
    n_params = len(in_names)
    all_in = tuple(in_names + out_names)
    donate = tuple(range(n_params, n_params + len(out_names)))

    def _body(*args):
        outs = bass2jax._bass_exec_p.bind(
            *args,
            out_avals=tuple(out_avals),
            in_names=all_in,
            out_names=tuple(out_names),
            lowering_input_output_aliases=(),
            sim_require_finite=False,
            sim_require_nnan=False,
            nc=nc,
        )
        return tuple(outs)

    jf = jax.jit(_body, donate_argnums=donate, keep_unused=True)
    compiled_devs: set = set()
    compile_lock = threading.Lock()

    def run(dev, in_map):
        args = [
            jax.device_put(np.ascontiguousarray(np.asarray(in_map[n])), dev)
            for n in in_names
        ]
        args += [
            jax.device_put(np.zeros(s, d), dev) for (s, d) in zero_specs
        ]
        if dev not in compiled_devs:
            # serialize first-time per-device XLA compiles
            with compile_lock:
                outs = jf(*args)
                jax.block_until_ready(outs)
                compiled_devs.add(dev)
        else:
            outs = jf(*args)
            jax.block_until_ready(outs)
        return {n: np.asarray(o) for n, o in zip(out_names, outs)}

    return run


# ------------------------------------------------------------ phase 1 (ctx)
def _build_phase1():
    """Per core: ctx[b,:] = mean_l enc[b,l,:] for its 8 batches.

    enc arrives fp8-e4m3 (host-cast).  The only error source is the fp8
    quantization of each element; the 2048-element mean averages it down
    (and PSUM accumulation of fp8 products is exact f32).  Each
    [128, 4096] tile holds 512 L-rows folded 4x into the free dim; a
    ones-vector matmul reduces the partition dim and PSUM accumulation
    folds both the tile and r axes.
    """
    nc = bacc.Bacc("TRN2", target_bir_lowering=False, debug=False, num_devices=1,
                   enable_partition_id=False)
    enc = nc.dram_tensor("enc", [B_LOC, L, D], FP8, kind="ExternalInput")
    ctx_out = nc.dram_tensor("ctx", [B_LOC, D], F32, kind="ExternalOutput")

    n_t = 4       # big tiles per batch
    n_r = 4       # L-rows folded per partition within a tile

    with tile.TileContext(nc) as tc, ExitStack() as st:
        tpool = st.enter_context(tc.tile_pool(name="enc", bufs=6))
        cpool = st.enter_context(tc.tile_pool(name="cst", bufs=1))
        ppool = st.enter_context(tc.tile_pool(name="ps", bufs=2, space="PSUM"))
        opool = st.enter_context(tc.tile_pool(name="row", bufs=2))

        ones = cpool.tile([128, 1], FP8)
        nc.gpsimd.memset(ones[:], 1.0)

        for b in range(B_LOC):
            # [2048, 1024] -> 4 tiles of [128, 4 * 1024]
            src = enc[b].rearrange("(t p r) d -> t p (r d)", p=128, r=n_r)
            ps = ppool.tile([1, D], F32)
            for t in range(n_t):
                et = tpool.tile([128, n_r * D], FP8)
                nc.sync.dma_start(et[:], src[t])
                for r in range(n_r):
                    for j in range(2):
                        c = r * D + j * 512
                        nc.tensor.matmul(
                            ps[:, j * 512:(j + 1) * 512], ones[:],
                            et[:, c:c + 512],
                            start=(t == 0 and r == 0),
                            stop=(t == n_t - 1 and r == n_r - 1))
            row = opool.tile([1, D], F32)
            nc.scalar.mul(row[:], ps[:], 1.0 / L)
            nc.sync.dma_start(ctx_out[b:b + 1, :], row[:])

    nc.compile()
    return nc


# ------------------------------------------------- phase 2 (GRU + proj)
def _build_phase2():
    """Per core: full-batch GRU step (replicated) + vocab-shard projection.

    All matmuls contract over the partition dim; activations stay f32.
    logits[:, c0:c1] = cat([h_new, ctx]) @ W_shard + out_b_shard via PSUM
    accumulation over 12 k-chunks plus a K=1 ones-row matmul for the bias.
    """
    nc = bacc.Bacc("TRN2", target_bir_lowering=False, debug=False, num_devices=1,
                   enable_partition_id=False)
    F32R = mybir.dt.float32r
    xeT = nc.dram_tensor("xeT", [E, B], BF16, kind="ExternalInput")
    ctxT = nc.dram_tensor("ctxT", [D, B], BF16, kind="ExternalInput")
    h0Tf = nc.dram_tensor("h0Tf", [H, B], F32, kind="ExternalInput")
    wihT = nc.dram_tensor("wihT", [K_CAT, 3 * H], BF16, kind="ExternalInput")
    whhT = nc.dram_tensor("whhT", [H, 3 * H], F32, kind="ExternalInput")
    brz = nc.dram_tensor("brz", [128, 8], F32, kind="ExternalInput")
    bin_ = nc.dram_tensor("bin", [128, 4], F32, kind="ExternalInput")
    bhn = nc.dram_tensor("bhn", [128, 4], F32, kind="ExternalInput")
    w_in = nc.dram_tensor("w", [K_CAT, VS], BF16, kind="ExternalInput")
    outb = nc.dram_tensor("outb", [1, VS], BF16, kind="ExternalInput")
    logits = nc.dram_tensor("logits", [B, VS], F32, kind="ExternalOutput")
    hnT_out = nc.dram_tensor("hnT", [H, B], F32, kind="ExternalOutput")

    KC = K_CAT // 128   # 12 cat-dim chunks
    KH = H // 128       # 4 hidden chunks

    with tile.TileContext(nc) as tc, ExitStack() as st:
        cpool = st.enter_context(tc.tile_pool(name="cst", bufs=1))
        gpool = st.enter_context(tc.tile_pool(name="gru", bufs=1))
        spool = st.enter_context(tc.tile_pool(name="sml", bufs=4))
        wpool = st.enter_context(tc.tile_pool(name="wts", bufs=10))
        lpool = st.enter_context(tc.tile_pool(name="lt", bufs=4))
        ps_g = st.enter_context(tc.tile_pool(name="psg", bufs=2, space="PSUM"))
        ps_p = st.enter_context(tc.tile_pool(name="psp", bufs=6, space="PSUM"))

        # --- constant / small loads
        ones1 = cpool.tile([1, B], BF16)
        nc.gpsimd.memset(ones1[:], 1.0)
        outb_sb = cpool.tile([1, VS], BF16)
        nc.sync.dma_start(outb_sb[:], outb[:])
        brz_sb = cpool.tile([128, 8], F32)
        nc.sync.dma_start(brz_sb[:], brz[:])
        bin_sb = cpool.tile([128, 4], F32)
        nc.sync.dma_start(bin_sb[:], bin_[:])
        bhn_sb = cpool.tile([128, 4], F32)
        nc.sync.dma_start(bhn_sb[:], bhn[:])

        # xt chunks: 0-3 = xe.T, 4-11 = ctx.T ; h0 chunks (all bf16)
        xt_sb = []
        for k in range(4):
            t = gpool.tile([128, B], BF16, tag=f"xt{k}")
            nc.sync.dma_start(t[:], xeT[k * 128:(k + 1) * 128, :])
            xt_sb.append(t)
        for k in range(8):
            t = gpool.tile([128, B], BF16, tag=f"ct{k}")
            nc.sync.dma_start(t[:], ctxT[k * 128:(k + 1) * 128, :])
            xt_sb.append(t)
        # h0 in f32: feeds both the (exact f32) gh matmuls and the
        # elementwise h_new update.
        h0f_sb = []
        for k in range(KH):
            t = gpool.tile([128, B], F32, tag=f"h0f{k}")
            nc.sync.dma_start(t[:], h0Tf[k * 128:(k + 1) * 128, :])
            h0f_sb.append(t)
        h0_sb = [t[:] for t in h0f_sb]

        wih_sb = []
        for k in range(KC):
            t = gpool.tile([128, 3 * H], BF16, tag=f"wi{k}")
            nc.sync.dma_start(t[:], wihT[k * 128:(k + 1) * 128, :])
            wih_sb.append(t)
        whh_sb = []
        for k in range(KH):
            t = gpool.tile([128, 3 * H], F32, tag=f"wh{k}")
            nc.sync.dma_start(t[:], whhT[k * 128:(k + 1) * 128, :])
            whh_sb.append(t)

        # --- GRU gates.  r/z: sigma(gi + gh + b); chunks g=0..7 of 3H rows.
        Sig = mybir.ActivationFunctionType.Sigmoid
        Ident = mybir.ActivationFunctionType.Identity
        TanhF = mybir.ActivationFunctionType.Tanh
        rz_sb = []
        for g in range(8):
            ps = ps_g.tile([128, B], F32, tag="psg")
            for k in range(KC):
                nc.tensor.matmul(ps[:], wih_sb[k][:, g * 128:(g + 1) * 128],
                                 xt_sb[k][:], start=(k == 0), stop=False)
            for k in range(KH):
                nc.tensor.matmul(
                    ps[:], whh_sb[k][:, g * 128:(g + 1) * 128],
                    h0_sb[k], start=False, stop=(k == KH - 1))
            act = gpool.tile([128, B], F32, tag=f"rz{g}")
            nc.scalar.activation(act[:], ps[:], Sig, bias=brz_sb[:, g:g + 1])
            rz_sb.append(act)

        # n chunks j=0..3 (rows 2H..3H) and h_new
        hn_new = []
        for j in range(KH):
            g = 8 + j
            ps_in = ps_g.tile([128, B], F32, tag="psg")
            for k in range(KC):
                nc.tensor.matmul(ps_in[:], wih_sb[k][:, g * 128:(g + 1) * 128],
                                 xt_sb[k][:], start=(k == 0), stop=(k == KC - 1))
            ps_hn = ps_g.tile([128, B], F32, tag="psg")
            for k in range(KH):
                nc.tensor.matmul(
                    ps_hn[:], whh_sb[k][:, g * 128:(g + 1) * 128],
                    h0_sb[k], start=(k == 0), stop=(k == KH - 1))
            in_sb = spool.tile([128, B], F32, tag="t_in")
            nc.scalar.activation(in_sb[:], ps_in[:], Ident,
                                 bias=bin_sb[:, j:j + 1])
            hn_sb = spool.tile([128, B], F32, tag="t_hn")
            nc.scalar.activation(hn_sb[:], ps_hn[:], Ident,
                                 bias=bhn_sb[:, j:j + 1])
            rhn = spool.tile([128, B], F32, tag="t_rhn")
            nc.vector.tensor_mul(rhn[:], rz_sb[j][:], hn_sb[:])
            pre_n = spool.tile([128, B], F32, tag="t_pre")
            nc.vector.tensor_add(pre_n[:], in_sb[:], rhn[:])
            n_sb = spool.tile([128, B], F32, tag="t_n")
            nc.scalar.activation(n_sb[:], pre_n[:], TanhF)
            d_sb = spool.tile([128, B], F32, tag="t_d")
            nc.vector.tensor_sub(d_sb[:], h0f_sb[j][:], n_sb[:])
            zd = spool.tile([128, B], F32, tag="t_zd")
            nc.vector.tensor_mul(zd[:], rz_sb[4 + j][:], d_sb[:])
            hnw = gpool.tile([128, B], F32, tag=f"hn{j}")
            nc.vector.tensor_add(hnw[:], n_sb[:], zd[:])
            nc.sync.dma_start(hnT_out[j * 128:(j + 1) * 128, :], hnw[:])
            hnb = gpool.tile([128, B], BF16, tag=f"hnb{j}")
            nc.vector.tensor_copy(hnb[:], hnw[:])
            hn_new.append(hnb)

        cat_sb = hn_new + xt_sb[4:]  # 12 bf16 chunks [128, B] = [h_new; ctx].T

        # --- vocab projection: bf16, 512-wide vtiles (one PSUM bank each),
        #     groups of <=6 (6 banks + 2 GRU = 8)
        bounds = [0, 2560, 5120, VS]
        groups = []
        for gi in range(len(bounds) - 1):
            c0, c1 = bounds[gi], bounds[gi + 1]
            vt = [(c0 + v * 512, min(c0 + (v + 1) * 512, c1))
                  for v in range((c1 - c0 + 511) // 512)]
            groups.append((c0, c1, vt))

        # Contract ctx chunks (k=4..11, ready at kernel start) before the
        # h_new chunks (k=0..3, ready only after the GRU) so the W stream
        # and PE stay busy while the GRU completes.
        k_order = list(range(4, KC)) + list(range(4))
        for (c0, c1, vtiles) in groups:
            gw = c1 - c0
            wk_tiles = {}
            for k in k_order:
                wk = wpool.tile([128, gw], BF16, tag="wk")
                nc.sync.dma_start(wk[:], w_in[k * 128:(k + 1) * 128, c0:c1])
                wk_tiles[k] = wk
            ps_list = []
            for (v0, v1) in vtiles:
                ps = ps_p.tile([B, 512], F32, tag="psp")
                ps_list.append(ps)
            for ki, k in enumerate(k_order):
                for i, (v0, v1) in enumerate(vtiles):
                    nc.tensor.matmul(
                        ps_list[i][:, 0:v1 - v0], cat_sb[k][:],
                        wk_tiles[k][:, v0 - c0:v1 - c0],
                        start=(ki == 0), stop=False)
            for i, (v0, v1) in enumerate(vtiles):
                nc.tensor.matmul(ps_list[i][:, 0:v1 - v0], ones1[:],
                                 outb_sb[:, v0:v1], start=False, stop=True)
                lt = lpool.tile([B, 512], F32, tag="lt")
                nc.vector.tensor_copy(lt[:, 0:v1 - v0], ps_list[i][:, 0:v1 - v0])
                nc.sync.dma_start(logits[:, v0:v1], lt[:, 0:v1 - v0])

    nc.compile()
    return nc


# ------------------------------------------------------------------ driver
def _get_state():
    import jax

    with _lock:
        if "p1" not in _state:
            _state["p1"] = _build_phase1()
            _state["run1"] = _make_runner(_state["p1"])
            _state["p2"] = _build_phase2()
            _state["run2"] = _make_runner(_state["p2"])
            # First transfer in a process pays a large one-time session
            # cost on the remote side; warm it with a tiny put per device.
            tiny = np.zeros(8, np.float32)
            for d in jax.devices()[:N_CORES]:
                jax.device_put(tiny, d).block_until_ready()
        return _state


def _run_cores(run, in_maps, devs):
    with ThreadPoolExecutor(N_CORES) as ex:
        futs = [ex.submit(run, devs[c], in_maps[c]) for c in range(N_CORES)]
        return [f.result() for f in futs]


def kernel(x, h, encoder_out, use_cuda=None, emb=None, attn_w=None,
           attn_b=None, w_ih=None, w_hh=None, b_ih=None, b_hh=None,
           out_w=None, out_b=None):
    import jax

    x = np.asarray(x)
    h = np.asarray(h, dtype=np.float32)
    encoder_out = np.ascontiguousarray(np.asarray(encoder_out, dtype=np.float32))
    emb = np.asarray(emb, dtype=np.float32)
    w_ih = np.asarray(w_ih, dtype=np.float32)
    w_hh = np.asarray(w_hh, dtype=np.float32)
    b_ih = np.asarray(b_ih, dtype=np.float32)
    b_hh = np.asarray(b_hh, dtype=np.float32)
    out_w = np.asarray(out_w, dtype=np.float32)
    out_b = np.asarray(out_b, dtype=np.float32)

    import ml_dtypes

    bf16 = ml_dtypes.bfloat16
    st = _get_state()
    devs = jax.devices()[:N_CORES]

    # ---------------- phase 1: context mean, batch-sharded (fp8 stream)
    enc_q = encoder_out.astype(ml_dtypes.float8_e4m3)
    in1 = [{"enc": enc_q[c * B_LOC:(c + 1) * B_LOC]} for c in range(N_CORES)]
    with _phase_ctx("p1"):
        res1 = _run_cores(st["run1"], in1, devs)
    context = np.concatenate([r["ctx"] for r in res1], axis=0)  # [B, D] f32

    # ---------------- host glue (the "all-gather")
    xe = emb[np.asarray(x[:, 0], dtype=np.int64)]               # [B, E]
    xeT = np.ascontiguousarray(xe.T).astype(bf16)
    h0Tf = np.ascontiguousarray(h[0].T)                         # [H, B] f32
    ctxT = np.ascontiguousarray(context.T).astype(bf16)         # [D, B]
    wihT = np.ascontiguousarray(w_ih.T).astype(bf16)            # [K_CAT, 3H]
    whhT = np.ascontiguousarray(w_hh.T)                         # [H, 3H] f32
    bsum = b_ih + b_hh
    brz = np.ascontiguousarray(bsum[:2 * H].reshape(8, 128).T)
    bin_ = np.ascontiguousarray(b_ih[2 * H:].reshape(4, 128).T)
    bhn = np.ascontiguousarray(b_hh[2 * H:].reshape(4, 128).T)

    w_pad = np.zeros((K_CAT, VP), dtype=bf16)
    w_pad[:, :V] = out_w.astype(bf16)
    b_pad = np.zeros((1, VP), dtype=bf16)
    b_pad[0, :V] = out_b.astype(bf16)

    # ---------------- phase 2: GRU + vocab-sharded projection
    in2 = []
    for c in range(N_CORES):
        sl = slice(c * VS, (c + 1) * VS)
        in2.append({
            "xeT": xeT, "h0T": h0Tf, "h0Tf": h0Tf, "ctxT": ctxT,
            "wihT": wihT, "whhT": whhT, "brz": brz, "bin": bin_, "bhn": bhn,
            "w": np.ascontiguousarray(w_pad[:, sl]),
            "outb": np.ascontiguousarray(b_pad[:, sl]),
        })
    with _phase_ctx("p2"):
        res2 = _run_cores(st["run2"], in2, devs)

    logits = np.concatenate([r["logits"] for r in res2], axis=1)[:, :V]
    h_new = np.ascontiguousarray(res2[0]["hnT"].T)              # [B, H]

    # ---------------- host epilogue: log_softmax normalization
    lg64 = logits.astype(np.float64)
    m = lg64.max(axis=1, keepdims=True)
    lse = (m + np.log(np.exp(lg64 - m).sum(axis=1, keepdims=True)))
    logp = (lg64 - lse).astype(np.float32)

    hidden = h_new[None]                                        # [1, B, H]
    attn_weights = np.full((B, 1, L), np.float32(1.0) / np.float32(L),
                           dtype=np.float32)
    return logp, hidden, attn_weights
